# revision 12
# baseline (speedup 1.0000x reference)
"""Trainium2 Bass kernel for sliding-window causal self-attention (GQA + RoPE +
RMS-norm QK + value-embedding gating).

Sharding: 8 cores = 2 (batch) x 4 (KV groups).  Each core handles one batch
element and one KV head (= 4 query heads), computes a partial output through
the row-slice of Wproj for its heads; the host sums the 4 partials per batch.
"""

import sys
import os

for _p in ("/root/.axon_site", "/root/.axon_site/_ro/trn_rl_repo",
           "/root/.axon_site/_ro/pypackages", "/opt/trn_rl_repo"):
    if os.path.isdir(_p) and _p not in sys.path:
        sys.path.append(_p)

import numpy as np
import ml_dtypes
from contextlib import ExitStack

import concourse.bass as bass
import concourse.tile as tile
from concourse import bacc, mybir
from concourse.bass_utils import run_bass_kernel_spmd

BF16 = ml_dtypes.bfloat16
N_HEAD, N_KV, HEAD_DIM, WINDOW, N_EMBD = 16, 4, 64, 512, 1024
B, T = 2, 2048
NCORES = 8
TCH = 512               # token chunk for the projection phase
NCH = T // TCH          # 4
NTT = T // 128          # 16 t-tiles
HPK = N_HEAD // N_KV    # 4 query heads per core

F32 = mybir.dt.float32
BF = mybir.dt.bfloat16
AF = mybir.ActivationFunctionType
OP = mybir.AluOpType

_cache = {}


def _build(debug_taps=False):
    nc = bacc.Bacc("TRN2", target_bir_lowering=False, debug=False,
                   num_devices=NCORES)

    xt_d = nc.dram_tensor("xt", [8, 128, T], BF, kind="ExternalInput")
    wq_d = nc.dram_tensor("wq", [8, 128, 256], BF, kind="ExternalInput")
    wkv_d = nc.dram_tensor("wkv", [8, 128, 128], BF, kind="ExternalInput")
    wg_d = nc.dram_tensor("wg", [32, 1], BF, kind="ExternalInput")
    wp_d = nc.dram_tensor("wp", [2, 128, 1024], BF, kind="ExternalInput")
    cs1_d = nc.dram_tensor("cs1", [128, T], F32, kind="ExternalInput")
    cs2_d = nc.dram_tensor("cs2", [128, T], F32, kind="ExternalInput")
    ve_d = nc.dram_tensor("ve2", [16, 128, 64], BF, kind="ExternalInput")
    msk_d = nc.dram_tensor("masks", [128, 256], BF, kind="ExternalInput")
    id_d = nc.dram_tensor("ident", [64, 64], BF, kind="ExternalInput")
    selq_d = nc.dram_tensor("selq", [128, 33], BF, kind="ExternalInput")
    on64_d = nc.dram_tensor("ones64", [64, 1], BF, kind="ExternalInput")
    on1x_d = nc.dram_tensor("ones1x64", [1, 64], BF, kind="ExternalInput")
    id1_d = nc.dram_tensor("id1", [1, 1], BF, kind="ExternalInput")
    out_d = nc.dram_tensor("out", [T, N_EMBD], F32, kind="ExternalOutput")
    if debug_taps:
        qdbg_d = nc.dram_tensor("qdbg", [4, 64, T], BF, kind="ExternalOutput")
        kdbg_d = nc.dram_tensor("kdbg", [64, T], BF, kind="ExternalOutput")
        vdbg_d = nc.dram_tensor("vdbg", [128, NTT * 65], BF, kind="ExternalOutput")

    with tile.TileContext(nc) as tc, ExitStack() as ctx:
        pers = ctx.enter_context(tc.tile_pool(name="pers", bufs=1))
        work = ctx.enter_context(tc.tile_pool(name="work", bufs=2))
        ptw = ctx.enter_context(tc.tile_pool(name="ptw", bufs=6))
        outw = ctx.enter_context(tc.tile_pool(name="outw", bufs=3))
        # PSUM pools (8 banks total):
        pb512 = ctx.enter_context(tc.tile_pool(name="pb512", bufs=2, space="PSUM"))
        pb128 = ctx.enter_context(tc.tile_pool(name="pb128", bufs=2, space="PSUM"))
        pbyx = ctx.enter_context(tc.tile_pool(name="pbyx", bufs=2, space="PSUM"))
        pbsm = ctx.enter_context(tc.tile_pool(name="pbsm", bufs=2, space="PSUM"))

        # ---- persistent SBUF loads ----
        xt_sb = pers.tile([128, 8 * T], BF, tag="xt")
        for kt in range(8):
            nc.sync.dma_start(xt_sb[:, kt * T:(kt + 1) * T], xt_d[kt])
        wq_sb = pers.tile([128, 8 * 256], BF, tag="wq")
        for kt in range(8):
            nc.sync.dma_start(wq_sb[:, kt * 256:(kt + 1) * 256], wq_d[kt])
        wkv_sb = pers.tile([128, 8 * 128], BF, tag="wkv")
        for kt in range(8):
            nc.sync.dma_start(wkv_sb[:, kt * 128:(kt + 1) * 128], wkv_d[kt])
        wg_sb = pers.tile([32, 1], BF, tag="wg")
        nc.sync.dma_start(wg_sb[:], wg_d[:])
        wp_sb = pers.tile([128, 2 * 1024], BF, tag="wp")
        for p in range(2):
            nc.sync.dma_start(wp_sb[:, p * 1024:(p + 1) * 1024], wp_d[p])
        cs1_sb = pers.tile([128, T], F32, tag="cs1")
        nc.sync.dma_start(cs1_sb[:], cs1_d[:])
        cs2_sb = pers.tile([128, T], F32, tag="cs2")
        nc.sync.dma_start(cs2_sb[:], cs2_d[:])
        ve_sb = pers.tile([128, 16 * 64], BF, tag="ve")
        for j in range(16):
            nc.sync.dma_start(ve_sb[:, j * 64:(j + 1) * 64], ve_d[j])
        msk_sb = pers.tile([128, 256], BF, tag="msk")
        nc.sync.dma_start(msk_sb[:], msk_d[:])
        id_sb = pers.tile([64, 64], BF, tag="ident")
        nc.sync.dma_start(id_sb[:], id_d[:])
        selq_sb = pers.tile([128, 33], BF, tag="selq")
        nc.sync.dma_start(selq_sb[:], selq_d[:])
        on64_sb = pers.tile([64, 1], BF, tag="on64")
        nc.sync.dma_start(on64_sb[:], on64_d[:])
        on1x_sb = pers.tile([1, 64], BF, tag="on1x")
        nc.sync.dma_start(on1x_sb[:], on1x_d[:])
        id1_sb = pers.tile([1, 1], BF, tag="id1")
        nc.sync.dma_start(id1_sb[:], id1_d[:])

        # ---- persistent intermediates ----
        qt_sb = [pers.tile([64, T], BF, tag=f"qt{h}", name=f"qt{h}")
                 for h in range(4)]          # Q^T per head
        kt_sb = pers.tile([64, T], BF, tag="kt")     # K^T
        vn_sb = pers.tile([128, NTT * 65], BF, tag="vn")  # V natural + ones col
        yt_sb = [pers.tile([128, T], BF, tag=f"yt{p}", name=f"yt{p}")
                 for p in range(2)]          # y^T, heads stacked
        rk_sb = pers.tile([128, NTT], F32, tag="rk")  # K rms recip, natural

        nc.vector.memset(vn_sb[:], 1.0)      # ones columns (col 64 of each group)
        biasq_sb = pers.tile([128, 1], F32, tag="biasq")
        nc.vector.memset(biasq_sb[:], 64e-6)
        biask_sb = pers.tile([1, 1], F32, tag="biask")
        nc.vector.memset(biask_sb[:], 1e-6)

        # =========== Phase 1: projections + RoPE + RMS + V prep ===========
        for ch in range(NCH):
            c0 = ch * TCH
            csl = slice(c0, c0 + TCH)

            def qk_head_ops(ps, rows, dst, is_q):
                """RoPE + RMS for psum rows [rows, rows+64*n) -> dst slice."""
                n = (128 - rows) // 64 if not is_q else 2
                # rope: A = ps * cs1, B = ps * cs2 (row-aligned trig tiles)
                nr = 128 if is_q else 64
                # A rows hh:    x1*cos   | B rows hh:    x2*sin (shifted up)
                # A rows hh+32: x1*sin(dn)| B rows hh+32: x2*cos
                A = work.tile([128, TCH], F32, tag="ropeA", name="ropeA")
                Bt = work.tile([128, TCH], F32, tag="ropeB", name="ropeB")
                ro = work.tile([128, TCH], F32, tag="rope", name="rope")
                for hh in range(0, nr, 64):
                    h1 = slice(hh, hh + 32)
                    h2 = slice(hh + 32, hh + 64)
                    nc.vector.tensor_mul(A[h1], ps[h1], cs1_sb[h1, csl])
                    nc.vector.tensor_mul(Bt[h1], ps[h2], cs1_sb[h2, csl])
                    nc.vector.tensor_mul(A[h2], ps[h1], cs2_sb[h1, csl])
                    nc.vector.tensor_mul(Bt[h2], ps[h2], cs2_sb[h2, csl])
                    nc.vector.tensor_sub(ro[h1], A[h1], Bt[h1])
                    nc.vector.tensor_add(ro[h2], A[h2], Bt[h2])
                # rms: recip = 1/sqrt(ss*sc + bias); Q folds the 1/8 score scale
                sq = work.tile([128, TCH], BF, tag="sq", name="sq")
                nc.scalar.square(sq[:nr], ro[0:nr])
                if is_q:
                    ss = pb512.tile([33, TCH], F32, tag="b512", name="ssq")
                    nc.tensor.matmul(ss[:], selq_sb[:], sq[:], start=True,
                                     stop=True)
                    bcps = pbyx.tile([128, TCH], F32, tag="yx", name="bcps")
                    for i in range(2):
                        r = 32 * i
                        srt = work.tile([1, TCH], F32, tag=f"srt{i}",
                                        name=f"srt{i}")
                        nc.scalar.activation(srt[:], ss[r:r + 1], AF.Sqrt,
                                             bias=biasq_sb[r:r + 1], scale=1.0)
                        rcpf = work.tile([1, TCH], F32, tag=f"rcpf{i}",
                                         name=f"rcpf{i}")
                        nc.vector.reciprocal(rcpf[:], srt[:])
                        rcp = work.tile([1, TCH], BF, tag=f"rcp{i}",
                                        name=f"rcp{i}")
                        nc.scalar.copy(rcp[:], rcpf[:])
                        nc.tensor.matmul(bcps[64 * i:64 * i + 64], on1x_sb[:],
                                         rcp[:], start=True, stop=True)
                    for i in range(2):
                        nc.vector.tensor_mul(dst[i][:, csl],
                                             ro[64 * i:64 * i + 64],
                                             bcps[64 * i:64 * i + 64])
                else:
                    # K^T stays unnormalized; rms recip folded into exp scale
                    nc.vector.tensor_copy(dst, ro[0:nr])
                    ss = pb512.tile([33, TCH], F32, tag="b512", name="ssk")
                    nc.tensor.matmul(ss[0:1], on64_sb[:], sq[0:64], start=True,
                                     stop=True)
                    srt = work.tile([1, TCH], F32, tag="srt0", name="srtk")
                    nc.scalar.activation(srt[:], ss[0:1], AF.Sqrt,
                                         bias=biask_sb[:], scale=1.0 / 64)
                    rcpkf = work.tile([1, TCH], F32, tag="rcpf0",
                                      name="rcpkf")
                    nc.vector.reciprocal(rcpkf[:], srt[:])
                    rcpk = work.tile([1, TCH], BF, tag="rcp0", name="rcpk")
                    nc.scalar.copy(rcpk[:], rcpkf[:])
                    for j in range(4):
                        rkp = pbsm.tile([128, 1], BF, tag="sm", name="rkp")
                        nc.tensor.transpose(
                            rkp[:], rcpk[:, j * 128:(j + 1) * 128], id1_sb[:])
                        tt = ch * 4 + j
                        nc.scalar.copy(rk_sb[:, tt:tt + 1], rkp[:])

            # Q pairs
            for p in range(2):
                psq = pb512.tile([128, TCH], F32, tag="b512", name="psq")
                for kt in range(8):
                    nc.tensor.matmul(
                        psq[:], wq_sb[:, kt * 256 + p * 128: kt * 256 + (p + 1) * 128],
                        xt_sb[:, kt * T + c0: kt * T + c0 + TCH],
                        start=(kt == 0), stop=(kt == 7))
                qk_head_ops(psq, 0, qt_sb[2 * p:2 * p + 2], True)

            # K | V^T
            pskv = pb512.tile([128, TCH], F32, tag="b512", name="pskv")
            for kt in range(8):
                nc.tensor.matmul(
                    pskv[:], wkv_sb[:, kt * 128:(kt + 1) * 128],
                    xt_sb[:, kt * T + c0: kt * T + c0 + TCH],
                    start=(kt == 0), stop=(kt == 7))
            qk_head_ops(pskv, 0, kt_sb[:, csl], False)
            vt_bf = work.tile([64, TCH], BF, tag="vt", name="vt")
            nc.scalar.copy(vt_bf[:], pskv[64:128])

            # V natural (+ gate * ve) per t-tile
            for j in range(4):
                t0 = c0 + j * 128
                tt = ch * 4 + j
                vtp = pbsm.tile([128, 64], BF, tag="sm", name="vtp")
                nc.tensor.transpose(vtp[:], vt_bf[:, j * 128:(j + 1) * 128],
                                    id_sb[:])
                gps = pbsm.tile([128, 64], F32, tag="sm", name="gps")
                nc.tensor.matmul(gps[:, 0:1], xt_sb[0:32, t0:t0 + 128],
                                 wg_sb[:], start=True, stop=True)
                g_sb = outw.tile([128, 1], F32, tag="g", name="g")
                nc.scalar.activation(g_sb[:], gps[:, 0:1], AF.Sigmoid)
                nc.vector.scalar_tensor_tensor(
                    vn_sb[:, tt * 65: tt * 65 + 64],
                    ve_sb[:, tt * 64:(tt + 1) * 64], g_sb[:], vtp[:],
                    op0=OP.mult, op1=OP.add)

        # ====== Phase 2+3: attention per q-tile, then output projection ======
        for qt in range(NTT):
            lo = max(0, qt - 4)
            for h in range(HPK):
                p, hh = h // 2, (h % 2) * 64
                q_ap = qt_sb[h][:, qt * 128:(qt + 1) * 128]
                yext = pbyx.tile([65, 128], F32, tag="yx", name="yext")
                for kt in range(lo, qt + 1):
                    stp = pb128.tile([128, 128], F32, tag="st", name="stp")
                    nc.tensor.matmul(stp[:],
                                     kt_sb[:, kt * 128:(kt + 1) * 128], q_ap,
                                     start=True, stop=True)
                    pt = ptw.tile([128, 128], BF, tag="pt", name="pt")
                    nc.scalar.activation(pt[:], stp[:], AF.Exp,
                                         scale=rk_sb[:, kt:kt + 1])
                    if kt == qt:
                        nc.vector.tensor_mul(pt[:], pt[:], msk_sb[:, 0:128])
                    elif kt == qt - 4:
                        nc.vector.tensor_mul(pt[:], pt[:], msk_sb[:, 128:256])
                    nc.tensor.matmul(yext[:],
                                     vn_sb[:, kt * 65: kt * 65 + 65], pt[:],
                                     start=(kt == lo), stop=(kt == qt))
                rrf = outw.tile([1, 128], F32, tag="rrf", name="rrf")
                nc.vector.reciprocal(rrf[:], yext[64:65, :])
                rr = outw.tile([1, 128], BF, tag="rr", name="rr")
                nc.scalar.copy(rr[:], rrf[:])
                bcq = pbsm.tile([64, 128], F32, tag="sm", name="bcq")
                nc.tensor.matmul(bcq[:], on1x_sb[:], rr[:], start=True,
                                 stop=True)
                bca = outw.tile([64, 128], BF, tag="bca", name="bca")
                nc.scalar.copy(bca[:], bcq[:])
                nc.vector.tensor_mul(
                    yt_sb[p][hh:hh + 64, qt * 128:(qt + 1) * 128],
                    yext[0:64, :], bca[:])

            if debug_taps and qt == NTT - 1:
                for h in range(4):
                    nc.sync.dma_start(qdbg_d[h], qt_sb[h][:])
                nc.sync.dma_start(kdbg_d[:], kt_sb[:])
                nc.sync.dma_start(vdbg_d[:], vn_sb[:])
            # output projection for this t-tile
            for cc in range(2):
                ops = pb512.tile([128, TCH], F32, tag="b512", name="ops")
                for p in range(2):
                    nc.tensor.matmul(
                        ops[:], yt_sb[p][:, qt * 128:(qt + 1) * 128],
                        wp_sb[:, p * 1024 + cc * 512: p * 1024 + cc * 512 + 512],
                        start=(p == 0), stop=(p == 1))
                o_sb = outw.tile([128, TCH], F32, tag="osb", name="osb")
                if cc == 0:
                    nc.scalar.copy(o_sb[:], ops[:])
                else:
                    nc.vector.tensor_copy(o_sb[:], ops[:])
                nc.sync.dma_start(
                    out_d[qt * 128:(qt + 1) * 128, cc * 512:(cc + 1) * 512],
                    o_sb[:])

    nc.compile()
    return nc


def _prep_inputs(x, ve, cos, sin, Wq, Wk, Wv, Wproj, Wgate):
    """Build the 8 per-core input maps (host-side sharding + layout prep)."""
    cosT = np.ascontiguousarray(cos.T).astype(np.float32)   # [32, T]
    sinT = np.ascontiguousarray(sin.T).astype(np.float32)
    cs1 = np.concatenate([cosT, sinT, cosT, sinT], 0)       # [128, T]
    cs2 = np.concatenate([sinT, cosT, sinT, cosT], 0)
    masks = np.concatenate([
        np.triu(np.ones((128, 128), np.float32)),           # causal (col>=row)
        np.tril(np.ones((128, 128), np.float32)),           # window (col<=row)
    ], 1).astype(BF16)
    ident = np.eye(64, dtype=BF16)
    selq = np.zeros((128, 33), np.float32)
    selq[0:64, 0] = 1.0
    selq[64:128, 32] = 1.0
    selq = selq.astype(BF16)
    ones64 = np.ones((64, 1), BF16)
    ones1x64 = np.ones((1, 64), BF16)
    id1 = np.ones((1, 1), BF16)

    xT = [np.ascontiguousarray(x[b].astype(BF16).T).reshape(8, 128, T)
          for b in range(B)]
    in_maps = []
    for c in range(NCORES):
        b, g = c // 4, c % 4
        wq_g = np.ascontiguousarray(
            Wq[:, g * 256:(g + 1) * 256]).astype(BF16).reshape(8, 128, 256)
        wkv_g = np.concatenate(
            [Wk[:, g * 64:(g + 1) * 64], Wv[:, g * 64:(g + 1) * 64]],
            1).astype(BF16).reshape(8, 128, 128)
        wg_g = np.ascontiguousarray(Wgate[:, g:g + 1]).astype(BF16)
        wp_g = np.ascontiguousarray(
            Wproj[g * 256:(g + 1) * 256, :]).astype(BF16).reshape(2, 128, 1024)
        ve_g = np.ascontiguousarray(
            2.0 * ve[b, :, g * 64:(g + 1) * 64]).astype(BF16).reshape(16, 128, 64)
        in_maps.append({
            "xt": xT[b], "wq": wq_g, "wkv": wkv_g, "wg": wg_g, "wp": wp_g,
            "cs1": cs1, "cs2": cs2, "ve2": ve_g, "masks": masks,
            "ident": ident, "selq": selq, "ones64": ones64,
            "ones1x64": ones1x64, "id1": id1,
        })
    return in_maps


def _run(inputs, trace=False, tmpdir=None):
    if "nc" not in _cache:
        _cache["nc"] = _build()
    nc = _cache["nc"]
    in_maps = _prep_inputs(**inputs)
    res = run_bass_kernel_spmd(nc, in_maps, list(range(NCORES)), trace=trace,
                               tmpdir=tmpdir)
    out = np.zeros((B, T, N_EMBD), np.float32)
    for c in range(NCORES):
        out[c // 4] += np.asarray(res.results[c]["out"], np.float32)
    return out, res


def kernel(**inputs):
    out, _ = _run(inputs)
    return out
